# revision 3
# baseline (speedup 1.0000x reference)
"""HyperGNN message-passing kernel (nn_Conv_13778255086166) for 8 TRN2 NeuronCores.

Reference computation:
    Xp    = X @ W                                   [N, 64]
    Xe_s  = segment_sum(Xp[vertex], edges, E);  cnt = segment_sum(1, edges, E)
    Ye    = (homo / max(cnt,1)) * Xe_s              [E, 64]   (mean aggregation * homo)
    att_s = segment_sum(homo[edges], vertex, N)
    Xv    = segment_sum(Ye[edges], vertex, N) / att_s
    out   = row_l2_normalize(Xp + Xv)

Distribution (graph parallelism per the sharding hint): the incidence list is
sharded by vertex range — core k owns nodes [k*12500, (k+1)*12500) and all
incidences whose vertex falls in that range.  Per core:

  phase 0: Xp slice = X_local @ W -> DRAM table XpD [12544, 64]
  phase 1: per 128-edge tile, dma_gather the Xp rows of the tile's
           (host-sorted, padded) incidence slots, and accumulate them into
           PSUM with TensorE one-hot matmuls (selection matrix built on DVE
           from slot offsets); a parallel ones-matmul accumulates cnt.
           -> local partial Eacc [25088, 65] ([sums | cnt])
  AllReduce(Eacc) over the 8 cores -> Ered
  Ze build: Ze[:, 0:64] = Ered[:, 0:64] * homo / max(cnt, 1); Ze[:, 64] = homo
           -> ZeF [25088, 128] (512B rows; cols 65:127 never read)
  phase 2: per 128-node tile, dma_gather ZeF rows of the vertex-sorted slots,
           one-hot matmul -> PSUM [128, 65] = [sum Ye | att_sum]; finalize
           Xv = S * recip(max(att, eps)); out = (Xp + Xv) * recip(rownorm)
           -> out slice [12544, 64]; host concatenates the 8 node slices.

All arithmetic (matmul, all segment sums, normalizations) runs on device.
The host only reorganizes the incidence lists (shard by vertex range, order
by segment, pad to fixed per-tile capacity) and formats index tensors —
schedule/layout preparation, not computation.
"""

import os
from dataclasses import dataclass

import numpy as np

import concourse.bacc as bacc
import concourse.mybir as mybir
import concourse.tile as tile
from concourse import bass_utils

F32 = mybir.dt.float32
I16 = mybir.dt.int16


@dataclass(frozen=True)
class Cfg:
    n_cores: int = 8
    N: int = 100000
    E: int = 25000
    cap1: int = 1536   # incidence slots per 128-edge tile per core (mult of 128)
    cap2: int = 3072   # incidence slots per 128-node tile per core (mult of 128)

    @property
    def npc(self):
        assert self.N % self.n_cores == 0
        return self.N // self.n_cores

    @property
    def npcp(self):  # padded, with at least one spare zero row
        return (self.npc + 1 + 127) // 128 * 128

    @property
    def ntiles(self):
        return self.npcp // 128

    @property
    def ep(self):
        return (self.E + 1 + 127) // 128 * 128

    @property
    def etiles(self):
        return self.ep // 128


def wrap_idx(idx: np.ndarray) -> np.ndarray:
    """int16 index layout for dma_gather: element j at [j%16, j//16],
    replicated across the 8 16-partition groups (one per Q7 cpu)."""
    s = idx.shape[0]
    assert s % 16 == 0
    w = np.ascontiguousarray(idx.astype(np.int16).reshape(-1, 16).T)
    return np.tile(w, (8, 1))


def prep_core_inputs(cfg: Cfg, k: int, X, W, homo, vertex, edges):
    """Host-side shard/sort/pad for core k (index/layout reorganization only)."""
    npc, npcp = cfg.npc, cfg.npcp
    sel = (vertex >= k * npc) & (vertex < (k + 1) * npc)
    v_l = (np.asarray(vertex)[sel] - k * npc).astype(np.int64)
    e_l = np.asarray(edges)[sel].astype(np.int64)

    def build(seg, other, tiles_n, cap, pad_gather):
        o = np.argsort(seg, kind="stable")
        s, g = seg[o], other[o]
        t_of = s >> 7
        counts = np.bincount(t_of, minlength=tiles_n)
        assert (counts <= cap).all(), (counts.max(), cap)
        starts = np.cumsum(counts) - counts
        rank = np.arange(len(s)) - starts[t_of]
        dest = t_of * cap + rank
        S = tiles_n * cap
        gi = np.full(S, pad_gather, np.int64)
        off = np.zeros(S, np.float32)
        val = np.zeros(S, np.float32)
        gi[dest] = g
        off[dest] = (s & 127).astype(np.float32)
        val[dest] = 1.0
        return gi, off, val

    # P1: segment by edge, gather by local vertex; pads gather zero row npc.
    g1, off1, val1 = build(e_l, v_l, cfg.etiles, cfg.cap1, pad_gather=npc)
    # P2: segment by local vertex, gather by edge; pads gather zero row E.
    g2, off2, _ = build(v_l, e_l, cfg.ntiles, cfg.cap2, pad_gather=cfg.E)

    def tilemaj_idx(gi, tiles_n, cap):
        w = np.stack([wrap_idx(gi[t * cap:(t + 1) * cap]) for t in range(tiles_n)])
        return np.ascontiguousarray(w)

    def tilemaj_f32(a, tiles_n, cap):
        return np.ascontiguousarray(
            a.reshape(tiles_n, cap // 128, 128).transpose(0, 2, 1))

    Xt = np.zeros((64, npcp), np.float32)
    Xt[:, :npc] = np.asarray(X)[k * npc:(k + 1) * npc].T

    homo_pad = np.zeros(cfg.ep, np.float32)
    homo_pad[:cfg.E] = np.asarray(homo)
    homo_t = np.ascontiguousarray(homo_pad.reshape(cfg.etiles, 128).T)

    iota = np.broadcast_to(np.arange(128, dtype=np.float32), (128, 128)).copy()

    return {
        "Xt": Xt,
        "W": np.asarray(W, dtype=np.float32),
        "homo_t": homo_t,
        "iota": iota,
        "g1": tilemaj_idx(g1, cfg.etiles, cfg.cap1),
        "off1": tilemaj_f32(off1, cfg.etiles, cfg.cap1),
        "val1": tilemaj_f32(val1, cfg.etiles, cfg.cap1),
        "g2": tilemaj_idx(g2, cfg.ntiles, cfg.cap2),
        "off2": tilemaj_f32(off2, cfg.ntiles, cfg.cap2),
    }


def build_nc(cfg: Cfg):
    c1 = cfg.cap1 // 128
    c2 = cfg.cap2 // 128
    nc = bacc.Bacc("TRN2", target_bir_lowering=False, debug=False,
                   num_devices=cfg.n_cores)

    xt_d = nc.dram_tensor("Xt", [64, cfg.npcp], F32, kind="ExternalInput")
    w_d = nc.dram_tensor("W", [64, 64], F32, kind="ExternalInput")
    homo_d = nc.dram_tensor("homo_t", [128, cfg.etiles], F32, kind="ExternalInput")
    iota_d = nc.dram_tensor("iota", [128, 128], F32, kind="ExternalInput")
    g1_d = nc.dram_tensor("g1", [cfg.etiles, 128, cfg.cap1 // 16], I16, kind="ExternalInput")
    off1_d = nc.dram_tensor("off1", [cfg.etiles, 128, c1], F32, kind="ExternalInput")
    val1_d = nc.dram_tensor("val1", [cfg.etiles, 128, c1], F32, kind="ExternalInput")
    g2_d = nc.dram_tensor("g2", [cfg.ntiles, 128, cfg.cap2 // 16], I16, kind="ExternalInput")
    off2_d = nc.dram_tensor("off2", [cfg.ntiles, 128, c2], F32, kind="ExternalInput")
    out_d = nc.dram_tensor("out", [cfg.npcp, 64], F32, kind="ExternalOutput")

    xp_d = nc.dram_tensor("XpD", [cfg.npcp, 64], F32, kind="Internal")
    eacc_d = nc.dram_tensor("EaccD", [cfg.ep, 65], F32, kind="Internal")
    ered_d = nc.dram_tensor("EredD", [cfg.ep, 65], F32, kind="Internal", addr_space="Shared")
    zef_d = nc.dram_tensor("ZeFD", [cfg.ep, 128], F32, kind="Internal")

    with tile.TileContext(nc) as tc:
        with (
            tc.tile_pool(name="const", bufs=1) as pc,
            tc.tile_pool(name="idx", bufs=4) as pidx,
            tc.tile_pool(name="gather", bufs=3) as pg,
            tc.tile_pool(name="onehot", bufs=4) as pm,
            tc.tile_pool(name="sbout", bufs=3) as po,
            tc.tile_pool(name="fin", bufs=4) as pf,
            tc.tile_pool(name="psum", bufs=2, space="PSUM") as pp,
        ):
            xt_sb = pc.tile([64, cfg.npcp], F32)
            nc.sync.dma_start(out=xt_sb[:], in_=xt_d[:])
            w_sb = pc.tile([64, 64], F32)
            nc.sync.dma_start(out=w_sb[:], in_=w_d[:])
            iota_sb = pc.tile([128, 128], F32)
            nc.sync.dma_start(out=iota_sb[:], in_=iota_d[:])
            homo_sb = pc.tile([128, cfg.etiles], F32)
            nc.sync.dma_start(out=homo_sb[:], in_=homo_d[:])

            # phase 0: Xp = X_local @ W
            for t in range(cfg.ntiles):
                ps = pp.tile([128, 64], F32, tag="ps0")
                nc.tensor.matmul(ps[:], lhsT=xt_sb[:, t * 128:(t + 1) * 128],
                                 rhs=w_sb[:], start=True, stop=True)
                xp_sb = po.tile([128, 64], F32, tag="xp0")
                nc.vector.tensor_copy(out=xp_sb[:], in_=ps[:])
                nc.sync.dma_start(out=xp_d[t * 128:(t + 1) * 128, :], in_=xp_sb[:])

            # phase 1: edge-tile accumulation
            for s in range(cfg.etiles):
                gi = pidx.tile([128, cfg.cap1 // 16], I16, tag="gi1")
                nc.sync.dma_start(out=gi[:], in_=g1_d[s])
                of = pidx.tile([128, c1], F32, tag="of1")
                nc.sync.dma_start(out=of[:], in_=off1_d[s])
                vl = pidx.tile([128, c1], F32, tag="vl1")
                nc.sync.dma_start(out=vl[:], in_=val1_d[s])
                g = pg.tile([128, c1, 64], F32, tag="g1")
                nc.gpsimd.dma_gather(g[:], xp_d[:], gi[:], cfg.cap1, cfg.cap1, 64,
                                     single_packet=False)
                ps = pp.tile([128, 64], F32, tag="ps1")
                psc = pp.tile([128, 1], F32, tag="ps1c")
                for j in range(c1):
                    mt = pm.tile([128, 128], F32, tag="mt1")
                    nc.vector.tensor_scalar(out=mt[:], in0=iota_sb[:],
                                            scalar1=of[:, j:j + 1], scalar2=None,
                                            op0=mybir.AluOpType.is_equal)
                    nc.tensor.matmul(ps[:], lhsT=mt[:], rhs=g[:, j, :],
                                     start=(j == 0), stop=(j == c1 - 1))
                    nc.tensor.matmul(psc[:], lhsT=mt[:], rhs=vl[:, j:j + 1],
                                     start=(j == 0), stop=(j == c1 - 1))
                acc = po.tile([128, 65], F32, tag="acc1")
                nc.vector.tensor_copy(out=acc[:, 0:64], in_=ps[:])
                nc.vector.tensor_copy(out=acc[:, 64:65], in_=psc[:])
                nc.sync.dma_start(out=eacc_d[s * 128:(s + 1) * 128, :], in_=acc[:])

            # AllReduce edge partials
            nc.gpsimd.collective_compute(
                "AllReduce", mybir.AluOpType.add,
                replica_groups=[list(range(cfg.n_cores))],
                ins=[eacc_d.ap()], outs=[ered_d.ap()],
            )

            # Ze build: [Ye | homo | zeros]
            for t in range(cfg.etiles):
                er = pf.tile([128, 65], F32, tag="er")
                nc.sync.dma_start(out=er[:], in_=ered_d[t * 128:(t + 1) * 128, :])
                cntm = pf.tile([128, 1], F32, tag="cntm")
                nc.vector.tensor_scalar_max(out=cntm[:], in0=er[:, 64:65], scalar1=1.0)
                rec = pf.tile([128, 1], F32, tag="rec")
                nc.vector.reciprocal(out=rec[:], in_=cntm[:])
                scale = pf.tile([128, 1], F32, tag="scale")
                nc.vector.tensor_tensor(out=scale[:], in0=rec[:],
                                        in1=homo_sb[:, t:t + 1],
                                        op=mybir.AluOpType.mult)
                z = po.tile([128, 128], F32, tag="z")
                nc.vector.memset(z[:, 64:128], 0.0)
                nc.vector.tensor_scalar_mul(out=z[:, 0:64], in0=er[:, 0:64],
                                            scalar1=scale[:])
                nc.vector.tensor_copy(out=z[:, 64:65], in_=homo_sb[:, t:t + 1])
                nc.sync.dma_start(out=zef_d[t * 128:(t + 1) * 128, :], in_=z[:])

            # phase 2: node-tile accumulation + finalize
            for s in range(cfg.ntiles):
                gi = pidx.tile([128, cfg.cap2 // 16], I16, tag="gi2")
                nc.sync.dma_start(out=gi[:], in_=g2_d[s])
                of = pidx.tile([128, c2], F32, tag="of2")
                nc.sync.dma_start(out=of[:], in_=off2_d[s])
                g = pg.tile([128, c2, 128], F32, tag="g2")
                nc.gpsimd.dma_gather(g[:], zef_d[:], gi[:], cfg.cap2, cfg.cap2, 128,
                                     single_packet=False)
                ps = pp.tile([128, 65], F32, tag="ps2")
                for j in range(c2):
                    mt = pm.tile([128, 128], F32, tag="mt2")
                    nc.vector.tensor_scalar(out=mt[:], in0=iota_sb[:],
                                            scalar1=of[:, j:j + 1], scalar2=None,
                                            op0=mybir.AluOpType.is_equal)
                    nc.tensor.matmul(ps[:, 0:65], lhsT=mt[:], rhs=g[:, j, 0:65],
                                     start=(j == 0), stop=(j == c2 - 1))
                attm = pf.tile([128, 1], F32, tag="attm")
                nc.vector.tensor_scalar_max(out=attm[:], in0=ps[:, 64:65], scalar1=1e-30)
                arec = pf.tile([128, 1], F32, tag="arec")
                nc.vector.reciprocal(out=arec[:], in_=attm[:])
                xp_sb = pf.tile([128, 64], F32, tag="xpl")
                nc.sync.dma_start(out=xp_sb[:], in_=xp_d[s * 128:(s + 1) * 128, :])
                o = pf.tile([128, 64], F32, tag="o")
                nc.vector.tensor_scalar_mul(out=o[:], in0=ps[:, 0:64], scalar1=arec[:])
                nc.vector.tensor_tensor(out=o[:], in0=o[:], in1=xp_sb[:],
                                        op=mybir.AluOpType.add)
                sq = pf.tile([128, 64], F32, tag="sq")
                nc.vector.tensor_tensor(out=sq[:], in0=o[:], in1=o[:],
                                        op=mybir.AluOpType.mult)
                rs = pf.tile([128, 1], F32, tag="rs")
                nc.vector.reduce_sum(out=rs[:], in_=sq[:], axis=mybir.AxisListType.X)
                rn = pf.tile([128, 1], F32, tag="rn")
                nc.scalar.sqrt(out=rn[:], in_=rs[:])
                rnm = pf.tile([128, 1], F32, tag="rnm")
                nc.vector.tensor_scalar_max(out=rnm[:], in0=rn[:], scalar1=1e-30)
                rrec = pf.tile([128, 1], F32, tag="rrec")
                nc.vector.reciprocal(out=rrec[:], in_=rnm[:])
                ot = po.tile([128, 64], F32, tag="ot")
                nc.vector.tensor_scalar_mul(out=ot[:], in0=o[:], scalar1=rrec[:])
                nc.sync.dma_start(out=out_d[s * 128:(s + 1) * 128, :], in_=ot[:])

    nc.compile()
    return nc


_NC_CACHE = {}


def kernel(**inputs) -> np.ndarray:
    """Full inputs in, full output out. Shards across 8 NeuronCores internally."""
    cfg = Cfg()
    X = np.asarray(inputs["X"], dtype=np.float32)
    W = np.asarray(inputs["W"], dtype=np.float32)
    homo = np.asarray(inputs["homo"], dtype=np.float32)
    vertex = np.asarray(inputs["vertex"])
    edges = np.asarray(inputs["edges"])
    assert X.shape == (cfg.N, 64) and homo.shape == (cfg.E,)

    key = cfg
    if key not in _NC_CACHE:
        _NC_CACHE[key] = build_nc(cfg)
    nc = _NC_CACHE[key]

    in_maps = [prep_core_inputs(cfg, k, X, W, homo, vertex, edges)
               for k in range(cfg.n_cores)]
    res = bass_utils.run_bass_kernel_spmd(
        nc, in_maps, core_ids=list(range(cfg.n_cores)),
        trace=bool(os.environ.get("KERNEL_TRACE")))
    global _LAST_RESULT
    _LAST_RESULT = res
    out = np.concatenate(
        [res.results[k]["out"][:cfg.npc] for k in range(cfg.n_cores)], axis=0)
    return out.astype(np.float32)



# revision 17
# speedup vs baseline: 1.7144x; 1.7144x over previous
"""HyperGNN message-passing kernel (nn_Conv_13778255086166) for 8 TRN2 NeuronCores.

Reference computation:
    Xp    = X @ W                                   [N, 64]
    Xe_s  = segment_sum(Xp[vertex], edges, E);  cnt = segment_sum(1, edges, E)
    Ze    = (homo / max(cnt,1)) * Xe_s              [E, 64]   (mean aggregation * homo)
    att_s = segment_sum(homo[edges], vertex, N)
    Xv    = segment_sum(Ze[edges], vertex, N) / att_s
    out   = row_l2_normalize(Xp + Xv)

Distribution (graph parallelism): incidences sharded by vertex range; core k
owns nodes [k*12500, (k+1)*12500).  Per core:

  phase 0: Xp = X_local @ W -> DRAM fp32 XpD [npcp, 64] (finalize) and a bf16
           gather table XpB [npcp, 128] (cols 0:64 features, col 64 = 1.0
           count marker, rest zero; the pad/zero rows have col 64 = 0).
  phase 1: per 128-edge tile, dma_gather the XpB rows of the (host-sorted,
           tail-padded) incidence slots; per 128-slot block build a bf16
           one-hot from the slot offsets (DVE is_equal; pad slots get offset
           255 -> all-zero rows) and accumulate onehot.T @ g[:, j, 0:65] into
           PSUM -> [sums | cnt].  Pad slots carry index -1, which the Q7
           gather ucode trims, so descriptor generation scales with the real
           incidence count.  Gathers round-robin over 4 SWDGE queues so up to
           4 descriptor generations run on distinct Q7 core pairs.
  AllReduce(Eacc bf16) in two chunks; chunk A overlaps the second half of
           phase 1.
  Ze build: ZeF[:, 0:64] = Ered * homo / max(cnt, 1); ZeF[:, 64] = homo
           (bf16 [ep, 128] rows).
  phase 2: per 128-node tile, same gather + one-hot matmul against ZeF ->
           PSUM [sum Ze | att_sum]; finalize Xv = S / max(att, eps);
           out = (Xp + Xv) / rownorm (Square+rowsum on the idle ACT engine).

All arithmetic (matmul, all segment sums, normalizations) runs on device.
The host only reorganizes the incidence lists (shard by vertex range, order
by segment, pad to fixed per-tile capacity) and formats index tensors —
schedule/layout preparation, not computation.
"""

import os
from dataclasses import dataclass

import ml_dtypes
import numpy as np

import concourse.bacc as bacc
import concourse.mybir as mybir
import concourse.tile as tile
from concourse import bass_utils

F32 = mybir.dt.float32
BF16 = mybir.dt.bfloat16
I16 = mybir.dt.int16
BF = ml_dtypes.bfloat16


@dataclass(frozen=True)
class Cfg:
    n_cores: int = 8
    N: int = 100000
    E: int = 25000
    cap1: int = 1536   # incidence slots per 128-edge tile per core (mult of 128)
    cap2: int = 2944   # incidence slots per 128-node tile per core (mult of 128)
    nqueues: int = 4   # SWDGE queues used for dma_gather round-robin
    split_ar: bool = True   # chunk the AllReduce to overlap phase 1
    trim_pads: bool = True  # -1 gather pads (Q7 trims trailing negatives)

    @property
    def npc(self):
        assert self.N % self.n_cores == 0
        return self.N // self.n_cores

    @property
    def npcp(self):  # padded, with at least one spare zero row
        return (self.npc + 1 + 127) // 128 * 128

    @property
    def ntiles(self):
        return self.npcp // 128

    @property
    def ep(self):
        return (self.E + 1 + 127) // 128 * 128

    @property
    def etiles(self):
        return self.ep // 128


def wrap_idx(idx: np.ndarray) -> np.ndarray:
    """int16 index layout for dma_gather: element j at [j%16, j//16],
    replicated across the 8 16-partition groups (one per Q7 cpu)."""
    s = idx.shape[0]
    assert s % 16 == 0
    w = np.ascontiguousarray(idx.astype(np.int16).reshape(-1, 16).T)
    return np.tile(w, (8, 1))


def prep_inputs(cfg: Cfg, X, W, homo, vertex, edges):
    """Host-side shard/sort/pad for all cores (index/layout reorganization
    only).  Returns (in_maps, nb1, nb2) where nb1/nb2 are the per-tile
    128-slot block counts shared across cores (max over cores)."""
    npc, npcp = cfg.npc, cfg.npcp
    vertex = np.asarray(vertex)
    edges = np.asarray(edges)

    def build_core(k):
        sel = (vertex >= k * npc) & (vertex < (k + 1) * npc)
        v_l = (vertex[sel] - k * npc).astype(np.int64)
        e_l = edges[sel].astype(np.int64)
        return v_l, e_l

    shards = [build_core(k) for k in range(cfg.n_cores)]

    def sort_one(seg, other, tiles_n, cap):
        o = np.argsort(seg, kind="stable")
        s, g = seg[o], other[o]
        t_of = s >> 7
        counts = np.bincount(t_of, minlength=tiles_n)
        assert (counts <= cap).all(), (counts.max(), cap)
        return s, g, t_of, counts

    sorted1 = [sort_one(e_l, v_l, cfg.etiles, cfg.cap1) for v_l, e_l in shards]
    sorted2 = [sort_one(v_l, e_l, cfg.ntiles, cfg.cap2) for v_l, e_l in shards]

    # shared per-tile block counts: max over cores, >= 1
    nb1 = np.maximum(1, -(-np.stack([c for _, _, _, c in sorted1]).max(0) // 128))
    nb2 = np.maximum(1, -(-np.stack([c for _, _, _, c in sorted2]).max(0) // 128))

    def build(sorted_sg, nb, cap, pad_gather):
        s, g, t_of, counts = sorted_sg
        starts = np.cumsum(counts) - counts
        rank = np.arange(len(s)) - starts[t_of]
        dest = t_of * cap + rank
        S = len(counts) * cap
        gi = np.full(S, -1, np.int64)
        off = np.full(S, 255.0, np.float32)
        # pad [count, nb*128) with the zero row so every core's trimmed
        # count equals nb*128 (must match decode-side ring accounting)
        ar = np.arange(cap)
        if cfg.trim_pads:
            live = ar[None, :] < (nb * 128)[:, None]
        else:
            live = np.ones((len(counts), cap), bool)
        gi[live.ravel()] = pad_gather
        gi[dest] = g
        off[dest] = (s & 127).astype(np.float32)
        return gi, off

    def tilemaj_idx(gi, tiles_n, cap):
        w = np.stack([wrap_idx(gi[t * cap:(t + 1) * cap]) for t in range(tiles_n)])
        return np.ascontiguousarray(w)

    def tilemaj_f32(a, tiles_n, cap):
        return np.ascontiguousarray(
            a.reshape(tiles_n, cap // 128, 128).transpose(0, 2, 1))

    homo_pad = np.zeros(cfg.ep, np.float32)
    homo_pad[:cfg.E] = np.asarray(homo)
    homo_t = np.ascontiguousarray(homo_pad.reshape(cfg.etiles, 128).T)

    iota = np.broadcast_to(
        np.arange(128, dtype=np.float32).astype(BF), (128, 128)).copy()

    mark = (np.arange(npcp).reshape(cfg.ntiles, 128).T < npc).astype(np.float32)
    mark = np.ascontiguousarray(mark)

    in_maps = []
    for k in range(cfg.n_cores):
        g1, off1 = build(sorted1[k], nb1, cfg.cap1, npc)
        g2, off2 = build(sorted2[k], nb2, cfg.cap2, cfg.E)
        Xt = np.zeros((64, npcp), np.float32)
        Xt[:, :npc] = np.asarray(X)[k * npc:(k + 1) * npc].T
        in_maps.append({
            "Xt": Xt,
            "W": np.asarray(W, dtype=np.float32),
            "homo_t": homo_t,
            "iota": iota,
            "mark": mark,
            "g1": tilemaj_idx(g1, cfg.etiles, cfg.cap1),
            "off1": tilemaj_f32(off1, cfg.etiles, cfg.cap1),
            "g2": tilemaj_idx(g2, cfg.ntiles, cfg.cap2),
            "off2": tilemaj_f32(off2, cfg.ntiles, cfg.cap2),
        })
    return in_maps, tuple(int(x) for x in nb1), tuple(int(x) for x in nb2)


def build_nc(cfg: Cfg, nb1, nb2):
    c1 = cfg.cap1 // 128
    c2 = cfg.cap2 // 128
    nc = bacc.Bacc("TRN2", target_bir_lowering=False, debug=False,
                   num_devices=cfg.n_cores, num_swdge_queues=cfg.nqueues)

    xt_d = nc.dram_tensor("Xt", [64, cfg.npcp], F32, kind="ExternalInput")
    w_d = nc.dram_tensor("W", [64, 64], F32, kind="ExternalInput")
    homo_d = nc.dram_tensor("homo_t", [128, cfg.etiles], F32, kind="ExternalInput")
    iota_d = nc.dram_tensor("iota", [128, 128], BF16, kind="ExternalInput")
    g1_d = nc.dram_tensor("g1", [cfg.etiles, 128, cfg.cap1 // 16], I16, kind="ExternalInput")
    off1_d = nc.dram_tensor("off1", [cfg.etiles, 128, c1], F32, kind="ExternalInput")
    g2_d = nc.dram_tensor("g2", [cfg.ntiles, 128, cfg.cap2 // 16], I16, kind="ExternalInput")
    off2_d = nc.dram_tensor("off2", [cfg.ntiles, 128, c2], F32, kind="ExternalInput")
    mark_d = nc.dram_tensor("mark", [128, cfg.ntiles], F32, kind="ExternalInput")
    out_d = nc.dram_tensor("out", [cfg.npcp, 64], F32, kind="ExternalOutput")

    xp_d = nc.dram_tensor("XpD", [cfg.npcp, 64], F32, kind="Internal")
    xpb_d = nc.dram_tensor("XpB", [cfg.npcp, 128], BF16, kind="Internal")
    eacc_d = nc.dram_tensor("EaccD", [cfg.ep, 65], BF16, kind="Internal")
    ered_d = nc.dram_tensor("EredD", [cfg.ep, 65], BF16, kind="Internal", addr_space="Shared")
    zef_d = nc.dram_tensor("ZeFD", [cfg.ep, 128], BF16, kind="Internal")

    qn = [0]

    def next_q():
        q = qn[0]
        qn[0] = (q + 1) % cfg.nqueues
        return q

    with tile.TileContext(nc) as tc:
        with (
            tc.tile_pool(name="const", bufs=1) as pc,
            tc.tile_pool(name="idx", bufs=8) as pidx,
            tc.tile_pool(name="gather", bufs=6) as pg,
            tc.tile_pool(name="onehot", bufs=6) as pm,
            tc.tile_pool(name="sbout", bufs=4) as po,
            tc.tile_pool(name="fin", bufs=4) as pf,
            tc.tile_pool(name="psum", bufs=2, space="PSUM") as pp,
        ):
            xt_sb = pc.tile([64, cfg.npcp], F32)
            nc.sync.dma_start(out=xt_sb[:], in_=xt_d[:])
            w_sb = pc.tile([64, 64], F32)
            nc.sync.dma_start(out=w_sb[:], in_=w_d[:])
            iota_sb = pc.tile([128, 128], BF16)
            nc.sync.dma_start(out=iota_sb[:], in_=iota_d[:])
            homo_sb = pc.tile([128, cfg.etiles], F32)
            nc.sync.dma_start(out=homo_sb[:], in_=homo_d[:])
            mark_sb = pc.tile([128, cfg.ntiles], F32)
            nc.sync.dma_start(out=mark_sb[:], in_=mark_d[:])

            # phase 0: Xp = X_local @ W; fp32 table + bf16 gather table
            for t in range(cfg.ntiles):
                ps = pp.tile([128, 64], F32, tag="ps0")
                nc.tensor.matmul(ps[:], lhsT=xt_sb[:, t * 128:(t + 1) * 128],
                                 rhs=w_sb[:], start=True, stop=True)
                xp_sb = po.tile([128, 64], F32, tag="xp0")
                nc.vector.tensor_copy(out=xp_sb[:], in_=ps[:])
                nc.sync.dma_start(out=xp_d[t * 128:(t + 1) * 128, :], in_=xp_sb[:])
                xb = po.tile([128, 128], BF16, tag="xb0")
                nc.vector.memset(xb[:, 65:128], 0.0)
                # count marker: 1.0 for real rows, 0.0 for pad/zero rows
                nc.vector.tensor_copy(out=xb[:, 64:65], in_=mark_sb[:, t:t + 1])
                nc.vector.tensor_copy(out=xb[:, 0:64], in_=ps[:])
                nc.sync.dma_start(out=xpb_d[t * 128:(t + 1) * 128, :], in_=xb[:])

            # phase 1: edge-tile accumulation (order: chunk A tiles then B)
            half = cfg.etiles // 2
            for s in range(cfg.etiles):
                gi = pidx.tile([128, cfg.cap1 // 16], I16, tag="gi1")
                nc.sync.dma_start(out=gi[:], in_=g1_d[s])
                of = pidx.tile([128, c1], F32, tag="of1")
                nc.sync.dma_start(out=of[:], in_=off1_d[s])
                n1 = (nb1[s] if cfg.trim_pads else c1)
                g = pg.tile([128, c1, 128], BF16, tag="g1")
                nc.gpsimd.dma_gather(g[:, 0:n1, :], xpb_d[:], gi[:],
                                     n1 * 128, n1 * 128,
                                     128, single_packet=False, queue_num=next_q())
                ps = pp.tile([128, 65], F32, tag="ps1")
                for j in range(n1):
                    mt = pm.tile([128, 128], BF16, tag="mt1")
                    nc.vector.tensor_scalar(out=mt[:], in0=iota_sb[:],
                                            scalar1=of[:, j:j + 1], scalar2=None,
                                            op0=mybir.AluOpType.is_equal)
                    nc.tensor.matmul(ps[:], lhsT=mt[:], rhs=g[:, j, 0:65],
                                     start=(j == 0), stop=(j == n1 - 1))
                acc = po.tile([128, 65], BF16, tag="acc1")
                nc.vector.tensor_copy(out=acc[:], in_=ps[:])
                nc.sync.dma_start(out=eacc_d[s * 128:(s + 1) * 128, :], in_=acc[:])
                if cfg.split_ar and s == half - 1:
                    # AllReduce chunk A overlaps the remaining phase-1 tiles
                    nc.gpsimd.collective_compute(
                        "AllReduce", mybir.AluOpType.add,
                        replica_groups=[list(range(cfg.n_cores))],
                        ins=[eacc_d[0:half * 128, :]],
                        outs=[ered_d[0:half * 128, :]],
                    )
            if cfg.split_ar:
                nc.gpsimd.collective_compute(
                    "AllReduce", mybir.AluOpType.add,
                    replica_groups=[list(range(cfg.n_cores))],
                    ins=[eacc_d[half * 128:, :]],
                    outs=[ered_d[half * 128:, :]],
                )
            else:
                nc.gpsimd.collective_compute(
                    "AllReduce", mybir.AluOpType.add,
                    replica_groups=[list(range(cfg.n_cores))],
                    ins=[eacc_d.ap()],
                    outs=[ered_d.ap()],
                )

            # Ze build: bf16 [Ye*homo | homo | zeros]
            for t in range(cfg.etiles):
                er = pf.tile([128, 65], BF16, tag="er")
                nc.sync.dma_start(out=er[:], in_=ered_d[t * 128:(t + 1) * 128, :])
                cntm = pf.tile([128, 1], F32, tag="cntm")
                nc.vector.tensor_scalar_max(out=cntm[:], in0=er[:, 64:65], scalar1=1.0)
                rec = pf.tile([128, 1], F32, tag="rec")
                nc.vector.reciprocal(out=rec[:], in_=cntm[:])
                scale = pf.tile([128, 1], F32, tag="scale")
                nc.vector.tensor_tensor(out=scale[:], in0=rec[:],
                                        in1=homo_sb[:, t:t + 1],
                                        op=mybir.AluOpType.mult)
                z = po.tile([128, 128], BF16, tag="z")
                nc.vector.memset(z[:, 65:128], 0.0)
                nc.vector.tensor_scalar_mul(out=z[:, 0:64], in0=er[:, 0:64],
                                            scalar1=scale[:])
                nc.vector.tensor_copy(out=z[:, 64:65], in_=homo_sb[:, t:t + 1])
                nc.sync.dma_start(out=zef_d[t * 128:(t + 1) * 128, :], in_=z[:])

            # phase 2: node-tile accumulation + finalize
            for s in range(cfg.ntiles):
                gi = pidx.tile([128, cfg.cap2 // 16], I16, tag="gi2")
                nc.sync.dma_start(out=gi[:], in_=g2_d[s])
                of = pidx.tile([128, c2], F32, tag="of2")
                nc.sync.dma_start(out=of[:], in_=off2_d[s])
                n2 = (nb2[s] if cfg.trim_pads else c2)
                g = pg.tile([128, c2, 128], BF16, tag="g2")
                nc.gpsimd.dma_gather(g[:, 0:n2, :], zef_d[:], gi[:],
                                     n2 * 128, n2 * 128,
                                     128, single_packet=False, queue_num=next_q())
                ps = pp.tile([128, 65], F32, tag="ps2")
                for j in range(n2):
                    mt = pm.tile([128, 128], BF16, tag="mt2")
                    nc.vector.tensor_scalar(out=mt[:], in0=iota_sb[:],
                                            scalar1=of[:, j:j + 1], scalar2=None,
                                            op0=mybir.AluOpType.is_equal)
                    nc.tensor.matmul(ps[:, 0:65], lhsT=mt[:], rhs=g[:, j, 0:65],
                                     start=(j == 0), stop=(j == n2 - 1))
                attm = pf.tile([128, 1], F32, tag="attm")
                nc.vector.tensor_scalar_max(out=attm[:], in0=ps[:, 64:65], scalar1=1e-30)
                arec = pf.tile([128, 1], F32, tag="arec")
                nc.vector.reciprocal(out=arec[:], in_=attm[:])
                xp_sb = pf.tile([128, 64], F32, tag="xpl")
                nc.sync.dma_start(out=xp_sb[:], in_=xp_d[s * 128:(s + 1) * 128, :])
                o = pf.tile([128, 64], F32, tag="o")
                nc.vector.tensor_scalar_mul(out=o[:], in0=ps[:, 0:64], scalar1=arec[:])
                nc.vector.tensor_tensor(out=o[:], in0=o[:], in1=xp_sb[:],
                                        op=mybir.AluOpType.add)
                sq = pf.tile([128, 64], F32, tag="sq")
                rs = pf.tile([128, 1], F32, tag="rs")
                nc.scalar.activation(out=sq[:], in_=o[:],
                                     func=mybir.ActivationFunctionType.Square,
                                     accum_out=rs[:])
                rn = pf.tile([128, 1], F32, tag="rn")
                nc.scalar.sqrt(out=rn[:], in_=rs[:])
                rnm = pf.tile([128, 1], F32, tag="rnm")
                nc.vector.tensor_scalar_max(out=rnm[:], in0=rn[:], scalar1=1e-30)
                rrec = pf.tile([128, 1], F32, tag="rrec")
                nc.vector.reciprocal(out=rrec[:], in_=rnm[:])
                ot = po.tile([128, 64], F32, tag="ot")
                nc.vector.tensor_scalar_mul(out=ot[:], in0=o[:], scalar1=rrec[:])
                nc.sync.dma_start(out=out_d[s * 128:(s + 1) * 128, :], in_=ot[:])

    nc.compile()
    return nc


_NC_CACHE = {}
_LAST_RESULT = None


def kernel(**inputs) -> np.ndarray:
    """Full inputs in, full output out. Shards across 8 NeuronCores internally."""
    X = np.asarray(inputs["X"], dtype=np.float32)
    W = np.asarray(inputs["W"], dtype=np.float32)
    homo = np.asarray(inputs["homo"], dtype=np.float32)
    vertex = np.asarray(inputs["vertex"])
    edges = np.asarray(inputs["edges"])
    cfg = Cfg(
        nqueues=int(os.environ.get("KERNEL_NQ", "4")),
        split_ar=os.environ.get("KERNEL_SPLIT_AR", "1") == "1",
        trim_pads=os.environ.get("KERNEL_TRIM", "1") == "1",
    )
    assert X.shape == (cfg.N, 64) and homo.shape == (cfg.E,)

    in_maps, nb1, nb2 = prep_inputs(cfg, X, W, homo, vertex, edges)
    key = (cfg, nb1, nb2)
    if key not in _NC_CACHE:
        _NC_CACHE[key] = build_nc(cfg, nb1, nb2)
    nc = _NC_CACHE[key]
    res = bass_utils.run_bass_kernel_spmd(
        nc, in_maps, core_ids=list(range(cfg.n_cores)),
        trace=bool(os.environ.get("KERNEL_TRACE")))
    global _LAST_RESULT
    _LAST_RESULT = res
    out = np.concatenate(
        [res.results[k]["out"][:cfg.npc] for k in range(cfg.n_cores)], axis=0)
    return out.astype(np.float32)
